# revision 1
# baseline (speedup 1.0000x reference)
"""DistMatch (retrieval_knn) Trainium2 kernel — 8-core SPMD, bbox-pruned.

Problem (per batch group b of 4): for each of 8192 query points (int coords
in [0,128)^3), find the 5 candidates (of 8192) with smallest clipped L2
distance (ties -> lowest index, exactly like jax.lax.top_k), and accumulate
sigmoid-gated, distance-weighted candidate features.

Sharding: data-parallel over groups x query halves — core c handles group
c//2 and half c%2 of that group's (k-d sorted) queries.

Method:
  * Exact integer algebra: key = d2 + (2*orig_idx+1)/32768 is computed
    bit-exactly by one K=18 bf16 matmul per <=512 candidate columns
    (integer decomposition of coords/norms into bf16-exact rows; fraction
    rows last exploit ascending-K PSUM accumulation). Any pair with
    d2 >= 256 has weight exactly 0, so fp32 rounding above d2=511 is
    harmless.
  * Host k-d sorts queries (tiles of 128) and candidates (chunks of 256)
    and drops (qtile, chunk) pairs whose bounding boxes are > 16 apart —
    those pairs cannot contribute. A rank-max slot schedule keeps the
    SPMD program identical across all cores.
  * VectorE InstMax extracts the top-8 (-key) per PSUM chunk; a merge pass
    + round/fraction-decode recovers d2 and the original candidate index.
  * GPSIMD dma_gather fetches the 5 matched 512B feature rows per query
    (pre-scaled by their sigmoid on host); fused scalar_tensor_tensor ops
    produce the weighted sum; host assembles the final concat output.
"""

import numpy as np
import ml_dtypes

B = 4
NA = 8192
NB = 8192
C = 112
CPAD = 128
TOPK = 5
NCORES = 8
QPC = NA // 2
CHW = 256  # candidate chunk width

BF16 = ml_dtypes.bfloat16
F32 = np.float32

_CACHE: dict = {}


# ---------------------------------------------------------------- host math
def _lhs_rows(ca):
    a = ca.astype(np.int64)
    ah, al = a >> 3, a & 7
    na2 = (a * a).sum(1)
    ma, ra = na2 >> 8, na2 & 255
    rows = np.zeros((18, a.shape[0]), np.float64)
    for d in range(3):
        rows[4 * d + 0] = ah[:, d]
        rows[4 * d + 1] = ah[:, d]
        rows[4 * d + 2] = al[:, d]
        rows[4 * d + 3] = al[:, d]
    rows[12] = ma
    rows[13] = ra
    rows[14:18] = 1.0
    return rows.astype(F32)


def _rhs_rows(cb):
    """[m,3] -> [18,m]; fraction rows encode the ORIGINAL candidate index."""
    b = cb.astype(np.int64)
    m = b.shape[0]
    bh, bl = b >> 3, b & 7
    nb2 = (b * b).sum(1)
    mb, rb = nb2 >> 8, nb2 & 255
    f = 2 * np.arange(m, dtype=np.int64) + 1
    fh, fl = f >> 6, f & 63
    r = np.zeros((18, m), np.float64)
    for d in range(3):
        r[4 * d + 0] = 128.0 * bh[:, d]
        r[4 * d + 1] = 16.0 * bl[:, d]
        r[4 * d + 2] = 16.0 * bh[:, d]
        r[4 * d + 3] = 2.0 * bl[:, d]
    r[12] = -256.0
    r[13] = -1.0
    r[14] = -256.0 * mb
    r[15] = -1.0 * rb
    r[16] = -(fh / 512.0)
    r[17] = -(fl / 32768.0)
    return r.astype(F32)


def _bf16(a):
    out = a.astype(BF16)
    assert np.array_equal(out.astype(F32), a)
    return out


def _scaled_feats(fb, w1, b1):
    fb = fb.astype(F32)
    z = fb @ w1.astype(F32) + b1.astype(F32)
    s = (1.0 / (1.0 + np.exp(-z, dtype=F32))).astype(F32)
    out = np.zeros((fb.shape[0], CPAD), F32)
    out[:, :C] = s * fb
    return out


def _kd_order(pts, leaf):
    out = []

    def rec(ids):
        if len(ids) <= leaf:
            out.append(ids)
            return
        p = pts[ids]
        dim = int(np.argmax(p.max(0) - p.min(0)))
        half = len(ids) // 2
        part = np.argpartition(p[:, dim], half)
        rec(ids[part[:half]])
        rec(ids[part[half:]])

    rec(np.arange(len(pts)))
    return np.concatenate(out)


def _plan_group(ca_g, cb_g):
    """k-d sort orders + per-half per-qtile surviving chunk lists."""
    pa = _kd_order(ca_g, 128)
    pb = _kd_order(cb_g, CHW)
    qa, qb = ca_g[pa], cb_g[pb]
    ct = qb.reshape(-1, CHW, 3)
    clo, chi = ct.min(1), ct.max(1)
    chunk_lists = []
    for h in range(2):
        qt = qa[h * QPC : (h + 1) * QPC].reshape(-1, 128, 3)
        qlo, qhi = qt.min(1), qt.max(1)
        lo = np.maximum(qlo[:, None, :], clo[None, :, :])
        hi = np.minimum(qhi[:, None, :], chi[None, :, :])
        gap = np.maximum(lo - hi, 0).astype(np.int64)
        keep = (gap**2).sum(-1) < 256
        chunk_lists.append([np.flatnonzero(keep[t]) for t in range(keep.shape[0])])
    return pa, pb, chunk_lists


def _make_caps(all_counts):
    nslots = len(all_counts[0])
    ranked = [sorted(c, reverse=True) for c in all_counts]
    return [max(1, max(r[t] for r in ranked)) for t in range(nslots)]


def _pack_core(chunks, caps, nch_total):
    order = np.argsort([-len(c) for c in chunks], kind="stable")
    slot_chunks = []
    for t, qt in enumerate(order):
        sel = list(chunks[qt])
        assert len(sel) <= caps[t]
        if len(sel) < caps[t]:
            selset = set(sel)
            pad = next(c for c in range(nch_total) if c not in selset)
            sel = sel + [pad] * (caps[t] - len(sel))
        slot_chunks.append(np.array(sel))
    return order, slot_chunks


# ---------------------------------------------------------------- device
def _build_program(nq, nb, caps, bqt=8):
    import concourse.tile as tile
    from concourse import bacc, mybir
    from concourse import library_config
    from concourse.tile_rust import add_dep_helper

    nqt = nq // 128
    nbt = nqt // bqt
    assert nqt % bqt == 0 and len(caps) == nqt
    f32, bf16, i16 = mybir.dt.float32, mybir.dt.bfloat16, mybir.dt.int16
    TWO23 = float(2.0**23)
    AL = mybir.AluOpType
    totcol = sum(c * CHW for c in caps)
    col_off = np.cumsum([0] + [c * CHW for c in caps])

    nc = bacc.Bacc("TRN2", target_bir_lowering=False, debug=False)
    LT1 = nc.dram_tensor("lt1", [18, nq], bf16, kind="ExternalInput")
    RS = nc.dram_tensor("rs", [18, totcol], bf16, kind="ExternalInput")
    FBP = nc.dram_tensor("fbp", [nb, CPAD], f32, kind="ExternalInput")
    TMP = nc.dram_tensor("tmp", [nq, C], f32, kind="ExternalOutput")

    with tile.TileContext(nc) as tc:
        with (
            tc.tile_pool(name="const", bufs=1) as constp,
            tc.tile_pool(name="rstr", bufs=3) as rstrp,
            tc.tile_pool(name="nk", bufs=3) as nkp,
            tc.tile_pool(name="cand", bufs=3) as candp,
            tc.tile_pool(name="small", bufs=2) as smallp,
            tc.tile_pool(name="gath", bufs=2) as gathp,
            tc.tile_pool(name="psum", bufs=2, space="PSUM") as psp,
            tc.tile_pool(name="dram", bufs=2, space="DRAM") as dramp,
        ):
            lib_inst = nc.gpsimd.load_library(library_config.mlp)

            lt1_sb = constp.tile([18, nq], bf16)
            nc.sync.dma_start(lt1_sb[:], LT1[:])

            for bt in range(nbt):
                top8 = smallp.tile([128, bqt, 8], f32, tag="top8")
                for q8 in range(bqt):
                    t = bt * bqt + q8
                    w_t = caps[t] * CHW
                    rsb = rstrp.tile([18, 8192], bf16, tag="rsb")
                    nc.sync.dma_start(
                        rsb[:, :w_t], RS[:, col_off[t] : col_off[t] + w_t]
                    )
                    nps_t = (w_t + 2047) // 2048
                    cand = candp.tile([128, 32], f32, tag="cand")
                    if nps_t * 8 < 32:
                        nc.vector.memset(cand[:], -1.0e9)
                    for h in range(nps_t):
                        pw = min(2048, w_t - h * 2048)
                        ps = psp.tile([128, 2048], f32, tag="ps")
                        for cc in range(0, pw, 512):
                            mw = min(512, pw - cc)
                            nc.tensor.matmul(
                                ps[:, cc : cc + mw],
                                lt1_sb[:, t * 128 : (t + 1) * 128],
                                rsb[:, h * 2048 + cc : h * 2048 + cc + mw],
                                start=True,
                                stop=True,
                            )
                        nk = nkp.tile([128, 2048], f32, tag="nk")
                        nc.scalar.copy(nk[:, :pw], ps[:, :pw])
                        nc.vector.max(cand[:, h * 8 : (h + 1) * 8], nk[:, :pw])
                    nc.vector.max(top8[:, q8, :], cand[:])

                tmp_v = TMP[:].rearrange(
                    "(bt qt q) c -> bt q qt c", bt=nbt, qt=bqt, q=128
                )[bt]

                # top8 = -(d2 + frac), frac in (0, 0.5): exact recovery
                t8 = top8[:].rearrange("p a b -> p (a b)")
                wide = [128, bqt * 8]
                r1t = smallp.tile(wide, f32, tag="r1t")
                nc.vector.tensor_scalar(r1t[:], t8, -1.0, TWO23, AL.mult, AL.add)
                rr = smallp.tile(wide, f32, tag="rr")  # = d2
                nc.vector.tensor_scalar(rr[:], r1t[:], -TWO23, 0.0, AL.add, AL.add)
                ttm = smallp.tile(wide, f32, tag="ttm")  # = -frac
                nc.vector.tensor_tensor(ttm[:], t8, rr[:], AL.add)
                jj = smallp.tile(wide, f32, tag="jj")  # = orig index
                nc.vector.tensor_scalar(jj[:], ttm[:], -16384.0, -0.5, AL.mult, AL.add)
                jc = smallp.tile([128, bqt, 8], f32, tag="jc")
                nc.vector.tensor_scalar(
                    jc[:].rearrange("p a b -> p (a b)"), jj[:], 0.0, float(nb - 1),
                    AL.max, AL.min,
                )
                sq = smallp.tile(wide, f32, tag="sq")
                nc.scalar.sqrt(sq[:], rr[:])
                wgt = smallp.tile([128, bqt, 8], f32, tag="wgt")
                nc.scalar.activation(
                    wgt[:].rearrange("p a b -> p (a b)"), sq[:],
                    mybir.ActivationFunctionType.Relu, bias=1.0, scale=-0.0625,
                )

                # index export: wrapped [16, nidx//16] DRAM image (i16)
                nidx = bqt * TOPK * 128
                jcc = smallp.tile([128, bqt * TOPK], f32, tag="jcc")
                nc.vector.tensor_copy(
                    jcc[:].rearrange("p (a b) -> p a b", a=bqt, b=TOPK),
                    jc[:, :, 0:TOPK],
                )
                ncol = nidx // 16
                idxd = dramp.tile([nidx], i16, tag="idxd")
                nc.gpsimd.dma_start(
                    out=idxd[:].rearrange(
                        "(ql i qh) -> qh ql i", ql=16, i=bqt * TOPK, qh=8
                    ),
                    in_=jcc[:],
                )
                idx_sb = gathp.tile([128, ncol], i16, tag="idx_sb")
                idxw = idxd[:].rearrange("(p c) -> p c", p=16)
                for rep in range(8):
                    nc.sync.dma_start(idx_sb[rep * 16 : (rep + 1) * 16, :], idxw)

                G = gathp.tile([128, bqt * TOPK, CPAD], f32, tag="G")
                g_inst = nc.gpsimd.dma_gather(
                    G[:], FBP[:], idx_sb[:], nidx, nidx, CPAD,
                    single_packet=False,
                )
                add_dep_helper(g_inst.ins, lib_inst.ins, True, "gather waits lib")

                acc = gathp.tile([128, bqt, C], f32, tag="acc")
                for q8 in range(bqt):
                    nc.vector.tensor_scalar(
                        acc[:, q8, :], G[:, q8 * TOPK, :C], wgt[:, q8, 0:1], 0.0,
                        AL.mult, AL.add,
                    )
                    for k in range(1, TOPK):
                        nc.vector.scalar_tensor_tensor(
                            acc[:, q8, :], G[:, q8 * TOPK + k, :C],
                            wgt[:, q8, k : k + 1], acc[:, q8, :],
                            AL.mult, AL.add,
                        )
                nc.sync.dma_start(tmp_v, acc[:])

    nc.compile()
    return nc


# ---------------------------------------------------------------- driver
def _prepare(coords_a, coords_b, feats_b, w1, b1):
    """Plan, build/compile (cached by caps), and produce per-core inputs.

    Returns (nc, in_maps, row_maps): row_maps[c] maps each output row of
    core c to its original query row within the core's group.
    """
    plans = [_plan_group(coords_a[g], coords_b[g]) for g in range(B)]
    all_counts = []
    for g in range(B):
        for h in range(2):
            all_counts.append([len(x) for x in plans[g][2][h]])
    caps = _make_caps(all_counts)

    key = tuple(caps)
    if _CACHE.get("key") != key:
        _CACHE["nc"] = _build_program(QPC, NB, caps)
        _CACHE["key"] = key
    nc = _CACHE["nc"]

    in_maps, row_maps = [], []
    for g in range(B):
        pa, pb, chunk_lists = plans[g]
        fbp = _scaled_feats(feats_b[g], w1, b1)
        rb_sorted = np.ascontiguousarray(_rhs_rows(coords_b[g])[:, pb])
        for h in range(2):
            qids = pa[h * QPC : (h + 1) * QPC]
            my_q = coords_a[g][qids]
            order, slot_chunks = _pack_core(chunk_lists[h], caps, NB // CHW)
            lt = _lhs_rows(my_q)
            lt_slots = np.concatenate(
                [lt[:, t * 128 : (t + 1) * 128] for t in order], axis=1
            )
            rs = np.concatenate(
                [rb_sorted[:, c0 * CHW : (c0 + 1) * CHW]
                 for sel in slot_chunks for c0 in sel],
                axis=1,
            )
            row_maps.append(
                np.concatenate([qids[t * 128 : (t + 1) * 128] for t in order])
            )
            in_maps.append(
                {
                    "lt1": _bf16(lt_slots),
                    "rs": _bf16(np.ascontiguousarray(rs)),
                    "fbp": fbp,
                }
            )
    return nc, in_maps, row_maps


def kernel(coords_a, coords_b, feats_a, feats_b, w1, b1):
    from concourse.bass_utils import run_bass_kernel_spmd

    coords_a = np.asarray(coords_a)
    coords_b = np.asarray(coords_b)
    feats_a = np.asarray(feats_a, dtype=F32)
    feats_b = np.asarray(feats_b, dtype=F32)
    w1 = np.asarray(w1, dtype=F32)
    b1 = np.asarray(b1, dtype=F32)

    nc, in_maps, row_maps = _prepare(coords_a, coords_b, feats_b, w1, b1)
    res = run_bass_kernel_spmd(nc, in_maps, core_ids=list(range(NCORES)))

    out = np.empty((B, NA, 2 * C), F32)
    out[:, :, :C] = feats_a
    for c in range(NCORES):
        g = c // 2
        out[g][row_maps[c], C:] = res.results[c]["tmp"]
    return out



# revision 5
# speedup vs baseline: 1.3097x; 1.3097x over previous
"""DistMatch (retrieval_knn) Trainium2 kernel — 8-core SPMD, bbox-pruned.

Problem (per batch group b of 4): for each of 8192 query points (int coords
in [0,128)^3), find the 5 candidates (of 8192) with smallest clipped L2
distance (ties -> lowest index, exactly like jax.lax.top_k), and accumulate
sigmoid-gated, distance-weighted candidate features.

Sharding: data-parallel over groups x query halves — core c handles group
c//2 and half c%2 of that group's (k-d sorted) queries.

Method:
  * Exact integer algebra: key = d2 + (2*orig_idx+1)/32768 is computed
    bit-exactly by one K=18 bf16 matmul per <=512 candidate columns
    (integer decomposition of coords/norms into bf16-exact rows).
  * Host k-d sorts queries (tiles of 128) and candidates (chunks of 256)
    and drops (qtile, chunk) pairs whose bounding boxes are > 16 apart.
    A rank-max slot schedule keeps the SPMD program identical across cores.
  * Software-pipelined device program: per bt-group, phase A (matmul +
    VectorE top-8 straight from PSUM), phase D (decode + on-chip i16 index
    wrap build + GPSIMD dma_gather launch), phase C of the PREVIOUS group
    (ScalarE weighted products + VectorE reduce + output DMA). The gather
    of group k overlaps the matmul/max of group k+1, keeping GPSIMD (the
    serial ~10ns/row descriptor generator) saturated.
  * Ranked slots are dealt round-robin to bt-groups so each group carries
    ~1/4 of the candidate columns.
"""

import numpy as np
import ml_dtypes

B = 4
NA = 8192
NB = 8192
C = 112
CPAD = 128
TOPK = 5
NCORES = 8
QPC = NA // 2
CHW = 256  # candidate chunk width

BF16 = ml_dtypes.bfloat16
F32 = np.float32

_CACHE: dict = {}


# ---------------------------------------------------------------- host math
def _lhs_rows(ca):
    a = ca.astype(np.int64)
    ah, al = a >> 3, a & 7
    na2 = (a * a).sum(1)
    ma, ra = na2 >> 8, na2 & 255
    rows = np.zeros((18, a.shape[0]), np.float64)
    for d in range(3):
        rows[4 * d + 0] = ah[:, d]
        rows[4 * d + 1] = ah[:, d]
        rows[4 * d + 2] = al[:, d]
        rows[4 * d + 3] = al[:, d]
    rows[12] = ma
    rows[13] = ra
    rows[14:18] = 1.0
    return rows.astype(F32)


def _rhs_rows(cb):
    """[m,3] -> [18,m]; fraction rows encode the ORIGINAL candidate index."""
    b = cb.astype(np.int64)
    m = b.shape[0]
    bh, bl = b >> 3, b & 7
    nb2 = (b * b).sum(1)
    mb, rb = nb2 >> 8, nb2 & 255
    f = 2 * np.arange(m, dtype=np.int64) + 1
    fh, fl = f >> 6, f & 63
    r = np.zeros((18, m), np.float64)
    for d in range(3):
        r[4 * d + 0] = 128.0 * bh[:, d]
        r[4 * d + 1] = 16.0 * bl[:, d]
        r[4 * d + 2] = 16.0 * bh[:, d]
        r[4 * d + 3] = 2.0 * bl[:, d]
    r[12] = -256.0
    r[13] = -1.0
    r[14] = -256.0 * mb
    r[15] = -1.0 * rb
    r[16] = -(fh / 512.0)
    r[17] = -(fl / 32768.0)
    return r.astype(F32)


def _bf16(a):
    out = a.astype(BF16)
    assert np.array_equal(out.astype(F32), a)
    return out


def _scaled_feats(fb, w1, b1):
    fb = fb.astype(F32)
    z = fb @ w1.astype(F32) + b1.astype(F32)
    s = (1.0 / (1.0 + np.exp(-z, dtype=F32))).astype(F32)
    out = np.zeros((fb.shape[0], CPAD), F32)
    out[:, :C] = s * fb
    return out


def _kd_order(pts, leaf):
    out = []

    def rec(ids):
        if len(ids) <= leaf:
            out.append(ids)
            return
        p = pts[ids]
        dim = int(np.argmax(p.max(0) - p.min(0)))
        half = len(ids) // 2
        part = np.argpartition(p[:, dim], half)
        rec(ids[part[:half]])
        rec(ids[part[half:]])

    rec(np.arange(len(pts)))
    return np.concatenate(out)


def _plan_group(ca_g, cb_g):
    """k-d sort orders + per-half per-qtile surviving chunk lists."""
    pa = _kd_order(ca_g, 128)
    pb = _kd_order(cb_g, CHW)
    qa, qb = ca_g[pa], cb_g[pb]
    ct = qb.reshape(-1, CHW, 3)
    clo, chi = ct.min(1), ct.max(1)
    chunk_lists = []
    for h in range(2):
        qt = qa[h * QPC : (h + 1) * QPC].reshape(-1, 128, 3)
        qlo, qhi = qt.min(1), qt.max(1)
        lo = np.maximum(qlo[:, None, :], clo[None, :, :])
        hi = np.minimum(qhi[:, None, :], chi[None, :, :])
        gap = np.maximum(lo - hi, 0).astype(np.int64)
        keep = (gap**2).sum(-1) < 256
        chunk_lists.append([np.flatnonzero(keep[t]) for t in range(keep.shape[0])])
    return pa, pb, chunk_lists


def _make_caps(all_counts):
    nslots = len(all_counts[0])
    ranked = [sorted(c, reverse=True) for c in all_counts]
    return [max(1, max(r[t] for r in ranked)) for t in range(nslots)]


def _pack_core(chunks, caps, nch_total):
    order = np.argsort([-len(c) for c in chunks], kind="stable")
    slot_chunks = []
    for t, qt in enumerate(order):
        sel = list(chunks[qt])
        assert len(sel) <= caps[t]
        if len(sel) < caps[t]:
            selset = set(sel)
            pad = next(c for c in range(nch_total) if c not in selset)
            sel = sel + [pad] * (caps[t] - len(sel))
        slot_chunks.append(np.array(sel))
    return order, slot_chunks


# ---------------------------------------------------------------- device
def _build_program(nq, nb, caps, bqt=8):
    import concourse.tile as tile
    from concourse import bacc, mybir
    from concourse import library_config
    from concourse.tile_rust import add_dep_helper

    nqt = nq // 128
    nbt = nqt // bqt
    assert nqt % bqt == 0 and len(caps) == nqt
    f32, bf16, i16 = mybir.dt.float32, mybir.dt.bfloat16, mybir.dt.int16
    TWO23 = float(2.0**23)
    AL = mybir.AluOpType
    AF = mybir.ActivationFunctionType
    totcol = sum(c * CHW for c in caps)
    col_off = np.cumsum([0] + [c * CHW for c in caps])
    # rank of (bt, j) under round-robin dealing: r = nbt*j + bt
    rank = lambda bt, j: nbt * j + bt
    wmax = max(caps) * CHW
    npsmax = (wmax + 2047) // 2048
    nidx = bqt * TOPK * 128  # gather rows per bt-group

    nc = bacc.Bacc(
        "TRN2", target_bir_lowering=False, debug=False, num_swdge_queues=2
    )
    LT1 = nc.dram_tensor("lt1", [18, nq], bf16, kind="ExternalInput")
    RS = nc.dram_tensor("rs", [18, totcol], bf16, kind="ExternalInput")
    FBP = nc.dram_tensor("fbp", [nb, CPAD], f32, kind="ExternalInput")
    TMP = nc.dram_tensor("tmp", [nq, C], f32, kind="ExternalOutput")

    with tile.TileContext(nc) as tc:
        with (
            tc.tile_pool(name="const", bufs=1) as constp,
            tc.tile_pool(name="rstr", bufs=3) as rstrp,
            tc.tile_pool(name="cand", bufs=2) as candp,
            tc.tile_pool(name="small", bufs=2) as smallp,
            tc.tile_pool(name="wrap", bufs=2) as wrapp,
            tc.tile_pool(name="gath", bufs=2) as gathp,
            tc.tile_pool(name="prod", bufs=2) as prodp,
            tc.tile_pool(name="acc", bufs=2) as accp,
            tc.tile_pool(name="psum", bufs=2, space="PSUM") as psp,
        ):
            lib_inst = nc.gpsimd.load_library(library_config.mlp)

            lt1_sb = constp.tile([18, nq], bf16)
            nc.sync.dma_start(lt1_sb[:], LT1[:])

            state = {}  # carries phase-C inputs from the previous bt

            def phase_a(bt):
                top8 = smallp.tile([128, bqt, 8], f32, tag="top8")
                for j in range(bqt):
                    r = rank(bt, j)
                    w_t = caps[r] * CHW
                    rsb = rstrp.tile([18, wmax], bf16, tag="rsb")
                    nc.sync.dma_start(
                        rsb[:, :w_t], RS[:, col_off[r] : col_off[r] + w_t]
                    )
                    nps_t = (w_t + 2047) // 2048
                    cand = candp.tile([128, npsmax * 8], f32, tag="cand")
                    if nps_t < npsmax:
                        nc.vector.memset(cand[:], -1.0e9)
                    for h in range(nps_t):
                        pw = min(2048, w_t - h * 2048)
                        ps = psp.tile([128, 2048], f32, tag="ps")
                        for cc in range(0, pw, 512):
                            mw = min(512, pw - cc)
                            nc.tensor.matmul(
                                ps[:, cc : cc + mw],
                                lt1_sb[:, r * 128 : (r + 1) * 128],
                                rsb[:, h * 2048 + cc : h * 2048 + cc + mw],
                                start=True,
                                stop=True,
                            )
                        # top-8 of the chunk straight from PSUM
                        nc.vector.max(cand[:, h * 8 : (h + 1) * 8], ps[:, :pw])
                    nc.vector.max(top8[:, j, :], cand[:])
                return top8

            def phase_d(bt, top8):
                # decode keys: top8 = -(d2 + frac), frac in (0, 0.5)
                t8 = top8[:].rearrange("p a b -> p (a b)")
                wide = [128, bqt * 8]
                r1t = smallp.tile(wide, f32, tag="r1t")
                nc.vector.tensor_scalar(r1t[:], t8, -1.0, TWO23, AL.mult, AL.add)
                rr = smallp.tile(wide, f32, tag="rr")  # = d2
                nc.vector.tensor_scalar(rr[:], r1t[:], -TWO23, 0.0, AL.add, AL.add)
                ttm = smallp.tile(wide, f32, tag="ttm")  # = -frac
                nc.vector.tensor_tensor(ttm[:], t8, rr[:], AL.add)
                jj = smallp.tile(wide, f32, tag="jj")  # = orig index
                nc.vector.tensor_scalar(
                    jj[:], ttm[:], -16384.0, -0.5, AL.mult, AL.add
                )
                jc = smallp.tile([128, bqt, 8], f32, tag="jc")
                nc.vector.tensor_scalar(
                    jc[:].rearrange("p a b -> p (a b)"), jj[:], 0.0, float(nb - 1),
                    AL.max, AL.min,
                )
                sq = smallp.tile(wide, f32, tag="sq")
                nc.scalar.sqrt(sq[:], rr[:])
                wgt = smallp.tile([128, bqt, 8], f32, tag="wgt")
                nc.scalar.activation(
                    wgt[:].rearrange("p a b -> p (a b)"), sq[:],
                    AF.Relu, bias=1.0, scale=-0.0625,
                )

                # i16 index image: value of (query q, slot s=j*5+k) must land
                # at wrap[q%16, s*8 + q//16], replicated to all 128 partitions.
                jci = smallp.tile([128, bqt, TOPK], i16, tag="jci")
                nc.vector.tensor_copy(jci[:], jc[:, :, 0:TOPK])
                wrap = wrapp.tile([128, nidx // 16], i16, tag="wrap")
                wrap3 = wrap[0:16, :].rearrange("p (s a) -> p s a", a=8)
                for a in range(8):
                    nc.scalar.dma_start(
                        wrap3[:, :, a],
                        jci[16 * a : 16 * (a + 1), :, :].rearrange(
                            "p s k -> p (s k)"
                        ),
                    )
                nc.scalar.dma_start(wrap[16:32, :], wrap[0:16, :])
                nc.scalar.dma_start(wrap[32:64, :], wrap[0:32, :])
                nc.scalar.dma_start(wrap[64:128, :], wrap[0:64, :])

                G = gathp.tile([128, bqt * TOPK, CPAD], f32, tag="G")
                g_inst = nc.gpsimd.dma_gather(
                    G[:], FBP[:], wrap[:], nidx, nidx, CPAD,
                    single_packet=False, queue_num=bt % 2,
                )
                add_dep_helper(g_inst.ins, lib_inst.ins, True, "gather waits lib")
                return G, wgt

            def phase_c(bt, G, wgt):
                acc = accp.tile([128, bqt, C], f32, tag="acc")
                for j in range(bqt):
                    prod = prodp.tile([128, TOPK, C], f32, tag="prod")
                    for k in range(TOPK):
                        nc.scalar.mul(
                            prod[:, k, :], G[:, j * TOPK + k, :C],
                            wgt[:, j, k : k + 1],
                        )
                    nc.vector.tensor_reduce(
                        acc[:, j, :],
                        prod[:].rearrange("p k c -> p c k"),
                        mybir.AxisListType.X,
                        AL.add,
                    )
                tmp_v = TMP[:].rearrange(
                    "(rj rb q) c -> rb q rj c", rj=bqt, rb=nbt, q=128
                )[bt]
                nc.scalar.dma_start(tmp_v, acc[:])

            for bt in range(nbt):
                top8 = phase_a(bt)
                d_out = phase_d(bt, top8)
                if state:
                    phase_c(bt - 1, *state.pop(bt - 1))
                state[bt] = d_out
            phase_c(nbt - 1, *state.pop(nbt - 1))

    nc.compile()
    return nc


# ---------------------------------------------------------------- driver
def _prepare(coords_a, coords_b, feats_b, w1, b1):
    """Plan, build/compile (cached by caps), and produce per-core inputs.

    Returns (nc, in_maps, row_maps): row_maps[c] maps each output row of
    core c to its original query row within the core's group.
    """
    plans = [_plan_group(coords_a[g], coords_b[g]) for g in range(B)]
    all_counts = []
    for g in range(B):
        for h in range(2):
            all_counts.append([len(x) for x in plans[g][2][h]])
    caps = _make_caps(all_counts)

    key = tuple(caps)
    if _CACHE.get("key") != key:
        _CACHE["nc"] = _build_program(QPC, NB, caps)
        _CACHE["key"] = key
    nc = _CACHE["nc"]

    in_maps, row_maps = [], []
    for g in range(B):
        pa, pb, chunk_lists = plans[g]
        fbp = _scaled_feats(feats_b[g], w1, b1)
        rb_sorted = np.ascontiguousarray(_rhs_rows(coords_b[g])[:, pb])
        for h in range(2):
            qids = pa[h * QPC : (h + 1) * QPC]
            my_q = coords_a[g][qids]
            order, slot_chunks = _pack_core(chunk_lists[h], caps, NB // CHW)
            lt = _lhs_rows(my_q)
            lt_slots = np.concatenate(
                [lt[:, t * 128 : (t + 1) * 128] for t in order], axis=1
            )
            rs = np.concatenate(
                [rb_sorted[:, c0 * CHW : (c0 + 1) * CHW]
                 for sel in slot_chunks for c0 in sel],
                axis=1,
            )
            row_maps.append(
                np.concatenate([qids[t * 128 : (t + 1) * 128] for t in order])
            )
            in_maps.append(
                {
                    "lt1": _bf16(lt_slots),
                    "rs": _bf16(np.ascontiguousarray(rs)),
                    "fbp": fbp,
                }
            )
    return nc, in_maps, row_maps


def kernel(coords_a, coords_b, feats_a, feats_b, w1, b1):
    from concourse.bass_utils import run_bass_kernel_spmd

    coords_a = np.asarray(coords_a)
    coords_b = np.asarray(coords_b)
    feats_a = np.asarray(feats_a, dtype=F32)
    feats_b = np.asarray(feats_b, dtype=F32)
    w1 = np.asarray(w1, dtype=F32)
    b1 = np.asarray(b1, dtype=F32)

    nc, in_maps, row_maps = _prepare(coords_a, coords_b, feats_b, w1, b1)
    res = run_bass_kernel_spmd(nc, in_maps, core_ids=list(range(NCORES)))

    out = np.empty((B, NA, 2 * C), F32)
    out[:, :, :C] = feats_a
    for c in range(NCORES):
        g = c // 2
        out[g][row_maps[c], C:] = res.results[c]["tmp"]
    return out


# revision 7
# speedup vs baseline: 1.3652x; 1.0424x over previous
"""DistMatch (retrieval_knn) Trainium2 kernel — 8-core SPMD, bbox-pruned.

Problem (per batch group b of 4): for each of 8192 query points (int coords
in [0,128)^3), find the 5 candidates (of 8192) with smallest clipped L2
distance (ties -> lowest index, exactly like jax.lax.top_k), and accumulate
sigmoid-gated, distance-weighted candidate features.

Sharding: data-parallel over groups x query halves — core c handles group
c//2 and half c%2 of that group's (k-d sorted) queries.

Method:
  * Exact integer algebra: key = d2 + (2*orig_idx+1)/32768 is computed
    bit-exactly by one K=18 bf16 matmul per <=512 candidate columns
    (integer decomposition of coords/norms into bf16-exact rows).
  * Host k-d sorts queries (tiles of 128) and candidates (chunks of 256)
    and drops (qtile, chunk) pairs whose bounding boxes are > 16 apart.
    A rank-max slot schedule keeps the SPMD program identical across cores.
  * Software-pipelined device program: per bt-group, phase A (matmul +
    VectorE top-8 straight from PSUM), phase D (decode + on-chip i16 index
    wrap build + GPSIMD dma_gather launch), phase C of the PREVIOUS group
    (ScalarE weighted products + VectorE reduce + output DMA). The gather
    of group k overlaps the matmul/max of group k+1, keeping GPSIMD (the
    serial ~10ns/row descriptor generator) saturated.
  * Ranked slots are dealt round-robin to bt-groups so each group carries
    ~1/4 of the candidate columns.
"""

import numpy as np
import ml_dtypes

B = 4
NA = 8192
NB = 8192
C = 112
CPAD = 128
TOPK = 5
NCORES = 8
QPC = NA // 2
CHW = 256  # candidate chunk width

BF16 = ml_dtypes.bfloat16
F32 = np.float32

_CACHE: dict = {}


# ---------------------------------------------------------------- host math
def _lhs_rows(ca):
    a = ca.astype(np.int64)
    ah, al = a >> 3, a & 7
    na2 = (a * a).sum(1)
    ma, ra = na2 >> 8, na2 & 255
    rows = np.zeros((18, a.shape[0]), np.float64)
    for d in range(3):
        rows[4 * d + 0] = ah[:, d]
        rows[4 * d + 1] = ah[:, d]
        rows[4 * d + 2] = al[:, d]
        rows[4 * d + 3] = al[:, d]
    rows[12] = ma
    rows[13] = ra
    rows[14:18] = 1.0
    return rows.astype(F32)


def _rhs_rows(cb):
    """[m,3] -> [18,m]; fraction rows encode the ORIGINAL candidate index."""
    b = cb.astype(np.int64)
    m = b.shape[0]
    bh, bl = b >> 3, b & 7
    nb2 = (b * b).sum(1)
    mb, rb = nb2 >> 8, nb2 & 255
    f = 2 * np.arange(m, dtype=np.int64) + 1
    fh, fl = f >> 6, f & 63
    r = np.zeros((18, m), np.float64)
    for d in range(3):
        r[4 * d + 0] = 128.0 * bh[:, d]
        r[4 * d + 1] = 16.0 * bl[:, d]
        r[4 * d + 2] = 16.0 * bh[:, d]
        r[4 * d + 3] = 2.0 * bl[:, d]
    r[12] = -256.0
    r[13] = -1.0
    r[14] = -256.0 * mb
    r[15] = -1.0 * rb
    r[16] = -(fh / 512.0)
    r[17] = -(fl / 32768.0)
    return r.astype(F32)


def _bf16(a):
    out = a.astype(BF16)
    assert np.array_equal(out.astype(F32), a)
    return out


def _scaled_feats(fb, w1, b1):
    fb = fb.astype(F32)
    z = fb @ w1.astype(F32) + b1.astype(F32)
    s = (1.0 / (1.0 + np.exp(-z, dtype=F32))).astype(F32)
    out = np.zeros((fb.shape[0], CPAD), F32)
    out[:, :C] = s * fb
    return out


def _kd_order(pts, leaf):
    out = []

    def rec(ids):
        if len(ids) <= leaf:
            out.append(ids)
            return
        p = pts[ids]
        dim = int(np.argmax(p.max(0) - p.min(0)))
        half = len(ids) // 2
        part = np.argpartition(p[:, dim], half)
        rec(ids[part[:half]])
        rec(ids[part[half:]])

    rec(np.arange(len(pts)))
    return np.concatenate(out)


def _plan_group(ca_g, cb_g):
    """k-d sort orders + per-half per-qtile surviving chunk lists."""
    pa = _kd_order(ca_g, 128)
    pb = _kd_order(cb_g, CHW)
    qa, qb = ca_g[pa], cb_g[pb]
    ct = qb.reshape(-1, CHW, 3)
    clo, chi = ct.min(1), ct.max(1)
    chunk_lists = []
    for h in range(2):
        qt = qa[h * QPC : (h + 1) * QPC].reshape(-1, 128, 3)
        qlo, qhi = qt.min(1), qt.max(1)
        lo = np.maximum(qlo[:, None, :], clo[None, :, :])
        hi = np.minimum(qhi[:, None, :], chi[None, :, :])
        gap = np.maximum(lo - hi, 0).astype(np.int64)
        keep = (gap**2).sum(-1) < 256
        chunk_lists.append([np.flatnonzero(keep[t]) for t in range(keep.shape[0])])
    return pa, pb, chunk_lists


def _make_caps(all_counts):
    nslots = len(all_counts[0])
    ranked = [sorted(c, reverse=True) for c in all_counts]
    return [max(1, max(r[t] for r in ranked)) for t in range(nslots)]


def _pack_core(chunks, caps, nch_total):
    order = np.argsort([-len(c) for c in chunks], kind="stable")
    slot_chunks = []
    for t, qt in enumerate(order):
        sel = list(chunks[qt])
        assert len(sel) <= caps[t]
        if len(sel) < caps[t]:
            selset = set(sel)
            pad = next(c for c in range(nch_total) if c not in selset)
            sel = sel + [pad] * (caps[t] - len(sel))
        slot_chunks.append(np.array(sel))
    return order, slot_chunks


# ---------------------------------------------------------------- device
def _build_program(nq, nb, caps, bqt=8):
    import concourse.tile as tile
    from concourse import bacc, mybir
    from concourse import library_config
    from concourse.tile_rust import add_dep_helper

    nqt = nq // 128
    nbt = nqt // bqt
    assert nqt % bqt == 0 and len(caps) == nqt
    f32, bf16, i16 = mybir.dt.float32, mybir.dt.bfloat16, mybir.dt.int16
    TWO23 = float(2.0**23)
    AL = mybir.AluOpType
    AF = mybir.ActivationFunctionType
    totcol = sum(c * CHW for c in caps)
    col_off = np.cumsum([0] + [c * CHW for c in caps])
    # rank of (bt, j) under round-robin dealing: r = nbt*j + bt
    rank = lambda bt, j: nbt * j + bt
    wmax = max(caps) * CHW
    npsmax = (wmax + 2047) // 2048
    nidx = bqt * TOPK * 128  # gather rows per bt-group

    nc = bacc.Bacc(
        "TRN2", target_bir_lowering=False, debug=False, num_swdge_queues=2
    )
    LT1 = nc.dram_tensor("lt1", [18, nq], bf16, kind="ExternalInput")
    RS = nc.dram_tensor("rs", [18, totcol], bf16, kind="ExternalInput")
    FBP = nc.dram_tensor("fbp", [nb, CPAD], f32, kind="ExternalInput")
    TMP = nc.dram_tensor("tmp", [nq, C], f32, kind="ExternalOutput")

    with tile.TileContext(nc) as tc:
        with (
            tc.tile_pool(name="const", bufs=1) as constp,
            tc.tile_pool(name="rstr", bufs=3) as rstrp,
            tc.tile_pool(name="cand", bufs=2) as candp,
            tc.tile_pool(name="small", bufs=3) as smallp,
            tc.tile_pool(name="wrap", bufs=2) as wrapp,
            tc.tile_pool(name="gath", bufs=3) as gathp,
            tc.tile_pool(name="prod", bufs=2) as prodp,
            tc.tile_pool(name="acc", bufs=2) as accp,
            tc.tile_pool(name="psum", bufs=2, space="PSUM") as psp,
        ):
            lib_inst = nc.gpsimd.load_library(library_config.mlp)

            lt1_sb = constp.tile([18, nq], bf16)
            nc.sync.dma_start(lt1_sb[:], LT1[:])

            state = {}  # carries phase-C inputs from the previous bt

            def phase_a(bt):
                top8 = smallp.tile([128, bqt, 8], f32, tag="top8")
                for j in range(bqt):
                    r = rank(bt, j)
                    w_t = caps[r] * CHW
                    rsb = rstrp.tile([18, wmax], bf16, tag="rsb")
                    nc.sync.dma_start(
                        rsb[:, :w_t], RS[:, col_off[r] : col_off[r] + w_t]
                    )
                    nps_t = (w_t + 2047) // 2048
                    cand = candp.tile([128, npsmax * 8], f32, tag="cand")
                    if nps_t < npsmax:
                        nc.vector.memset(cand[:], -1.0e9)
                    for h in range(nps_t):
                        pw = min(2048, w_t - h * 2048)
                        ps = psp.tile([128, 2048], f32, tag="ps")
                        for cc in range(0, pw, 512):
                            mw = min(512, pw - cc)
                            nc.tensor.matmul(
                                ps[:, cc : cc + mw],
                                lt1_sb[:, r * 128 : (r + 1) * 128],
                                rsb[:, h * 2048 + cc : h * 2048 + cc + mw],
                                start=True,
                                stop=True,
                            )
                        # top-8 of the chunk straight from PSUM
                        nc.vector.max(cand[:, h * 8 : (h + 1) * 8], ps[:, :pw])
                    nc.vector.max(top8[:, j, :], cand[:])
                return top8

            def phase_d(bt, top8):
                # decode keys: top8 = -(d2 + frac), frac in (0, 0.5)
                t8 = top8[:].rearrange("p a b -> p (a b)")
                wide = [128, bqt * 8]
                r1t = smallp.tile(wide, f32, tag="r1t")
                nc.vector.tensor_scalar(r1t[:], t8, -1.0, TWO23, AL.mult, AL.add)
                rr = smallp.tile(wide, f32, tag="rr")  # = d2
                nc.vector.tensor_scalar(rr[:], r1t[:], -TWO23, 0.0, AL.add, AL.add)
                ttm = smallp.tile(wide, f32, tag="ttm")  # = -frac
                nc.vector.tensor_tensor(ttm[:], t8, rr[:], AL.add)
                jj = smallp.tile(wide, f32, tag="jj")  # = orig index
                nc.vector.tensor_scalar(
                    jj[:], ttm[:], -16384.0, -0.5, AL.mult, AL.add
                )
                jc = smallp.tile([128, bqt, 8], f32, tag="jc")
                nc.vector.tensor_scalar(
                    jc[:].rearrange("p a b -> p (a b)"), jj[:], 0.0, float(nb - 1),
                    AL.max, AL.min,
                )
                sq = smallp.tile(wide, f32, tag="sq")
                nc.scalar.sqrt(sq[:], rr[:])
                wgt = smallp.tile([128, bqt, 8], f32, tag="wgt")
                nc.scalar.activation(
                    wgt[:].rearrange("p a b -> p (a b)"), sq[:],
                    AF.Relu, bias=1.0, scale=-0.0625,
                )

                # i16 index image: value of (query q, slot s=j*5+k) must land
                # at wrap[q%16, s*8 + q//16], replicated to all 128 partitions.
                jci = smallp.tile([128, bqt, TOPK], i16, tag="jci")
                nc.vector.tensor_copy(jci[:], jc[:, :, 0:TOPK])
                wrap = wrapp.tile([128, nidx // 16], i16, tag="wrap")
                wrap3 = wrap[0:16, :].rearrange("p (s a) -> p s a", a=8)
                for a in range(8):
                    nc.scalar.dma_start(
                        wrap3[:, :, a],
                        jci[16 * a : 16 * (a + 1), :, :].rearrange(
                            "p s k -> p (s k)"
                        ),
                    )
                nc.scalar.dma_start(wrap[16:32, :], wrap[0:16, :])
                nc.scalar.dma_start(wrap[32:64, :], wrap[0:32, :])
                nc.scalar.dma_start(wrap[64:128, :], wrap[0:64, :])

                G = gathp.tile([128, bqt * TOPK, CPAD], f32, tag="G")
                g_inst = nc.gpsimd.dma_gather(
                    G[:], FBP[:], wrap[:], nidx, nidx, CPAD,
                    single_packet=False, queue_num=bt % 2,
                )
                add_dep_helper(g_inst.ins, lib_inst.ins, True, "gather waits lib")
                return G, wgt

            def phase_c(bt, G, wgt):
                acc = accp.tile([128, bqt, C], f32, tag="acc")
                for j in range(bqt):
                    prod = prodp.tile([128, TOPK, C], f32, tag="prod")
                    for k in range(TOPK):
                        nc.scalar.mul(
                            prod[:, k, :], G[:, j * TOPK + k, :C],
                            wgt[:, j, k : k + 1],
                        )
                    nc.vector.tensor_reduce(
                        acc[:, j, :],
                        prod[:].rearrange("p k c -> p c k"),
                        mybir.AxisListType.X,
                        AL.add,
                    )
                tmp_v = TMP[:].rearrange(
                    "(rj rb q) c -> rb q rj c", rj=bqt, rb=nbt, q=128
                )[bt]
                nc.scalar.dma_start(tmp_v, acc[:])

            for bt in range(nbt):
                top8 = phase_a(bt)
                state[bt] = phase_d(bt, top8)
                if bt >= 2:
                    phase_c(bt - 2, *state.pop(bt - 2))
            phase_c(nbt - 2, *state.pop(nbt - 2))
            phase_c(nbt - 1, *state.pop(nbt - 1))

    nc.compile()
    return nc


# ---------------------------------------------------------------- driver
def _prepare(coords_a, coords_b, feats_b, w1, b1):
    """Plan, build/compile (cached by caps), and produce per-core inputs.

    Returns (nc, in_maps, row_maps): row_maps[c] maps each output row of
    core c to its original query row within the core's group.
    """
    plans = [_plan_group(coords_a[g], coords_b[g]) for g in range(B)]
    all_counts = []
    for g in range(B):
        for h in range(2):
            all_counts.append([len(x) for x in plans[g][2][h]])
    caps = _make_caps(all_counts)

    key = tuple(caps)
    if _CACHE.get("key") != key:
        _CACHE["nc"] = _build_program(QPC, NB, caps)
        _CACHE["key"] = key
    nc = _CACHE["nc"]

    in_maps, row_maps = [], []
    for g in range(B):
        pa, pb, chunk_lists = plans[g]
        fbp = _scaled_feats(feats_b[g], w1, b1)
        rb_sorted = np.ascontiguousarray(_rhs_rows(coords_b[g])[:, pb])
        for h in range(2):
            qids = pa[h * QPC : (h + 1) * QPC]
            my_q = coords_a[g][qids]
            order, slot_chunks = _pack_core(chunk_lists[h], caps, NB // CHW)
            lt = _lhs_rows(my_q)
            lt_slots = np.concatenate(
                [lt[:, t * 128 : (t + 1) * 128] for t in order], axis=1
            )
            rs = np.concatenate(
                [rb_sorted[:, c0 * CHW : (c0 + 1) * CHW]
                 for sel in slot_chunks for c0 in sel],
                axis=1,
            )
            row_maps.append(
                np.concatenate([qids[t * 128 : (t + 1) * 128] for t in order])
            )
            in_maps.append(
                {
                    "lt1": _bf16(lt_slots),
                    "rs": _bf16(np.ascontiguousarray(rs)),
                    "fbp": fbp,
                }
            )
    return nc, in_maps, row_maps


def kernel(coords_a, coords_b, feats_a, feats_b, w1, b1):
    from concourse.bass_utils import run_bass_kernel_spmd

    coords_a = np.asarray(coords_a)
    coords_b = np.asarray(coords_b)
    feats_a = np.asarray(feats_a, dtype=F32)
    feats_b = np.asarray(feats_b, dtype=F32)
    w1 = np.asarray(w1, dtype=F32)
    b1 = np.asarray(b1, dtype=F32)

    nc, in_maps, row_maps = _prepare(coords_a, coords_b, feats_b, w1, b1)
    res = run_bass_kernel_spmd(nc, in_maps, core_ids=list(range(NCORES)))

    out = np.empty((B, NA, 2 * C), F32)
    out[:, :, :C] = feats_a
    for c in range(NCORES):
        g = c // 2
        out[g][row_maps[c], C:] = res.results[c]["tmp"]
    return out


# revision 9
# speedup vs baseline: 1.6832x; 1.2330x over previous
"""DistMatch (retrieval_knn) Trainium2 kernel — 8-core SPMD, bbox-pruned.

Problem (per batch group b of 4): for each of 8192 query points (int coords
in [0,128)^3), find the 5 candidates (of 8192) with smallest clipped L2
distance (ties -> lowest index, exactly like jax.lax.top_k), and accumulate
sigmoid-gated, distance-weighted candidate features.

Sharding: data-parallel over groups x query halves — core c handles group
c//2 and half c%2 of that group's (k-d sorted) queries.

Method:
  * Exact integer algebra: key = d2 + (2*orig_idx+1)/32768 is computed
    bit-exactly by one K=18 bf16 matmul per <=512 candidate columns
    (integer decomposition of coords/norms into bf16-exact rows).
  * Host k-d sorts queries (tiles of 128) and candidates (chunks of 256)
    and drops (qtile, chunk) pairs whose bounding boxes are > 16 apart.
    A rank-max slot schedule keeps the SPMD program identical across cores.
  * Software-pipelined device program: per bt-group, phase A (matmul +
    VectorE top-8 straight from PSUM), phase D (decode + on-chip i16 index
    wrap build + GPSIMD dma_gather launch), phase C of the PREVIOUS group
    (ScalarE weighted products + VectorE reduce + output DMA). The gather
    of group k overlaps the matmul/max of group k+1, keeping GPSIMD (the
    serial ~10ns/row descriptor generator) saturated.
  * Ranked slots are dealt round-robin to bt-groups so each group carries
    ~1/4 of the candidate columns.
"""

import numpy as np
import ml_dtypes

B = 4
NA = 8192
NB = 8192
C = 112
CPAD = 128
TOPK = 5
NCORES = 8
QPC = NA // 2
CHW = 256  # candidate chunk width

BF16 = ml_dtypes.bfloat16
F32 = np.float32

_CACHE: dict = {}


# ---------------------------------------------------------------- host math
def _lhs_rows(ca):
    a = ca.astype(np.int64)
    ah, al = a >> 3, a & 7
    na2 = (a * a).sum(1)
    ma, ra = na2 >> 8, na2 & 255
    rows = np.zeros((18, a.shape[0]), np.float64)
    for d in range(3):
        rows[4 * d + 0] = ah[:, d]
        rows[4 * d + 1] = ah[:, d]
        rows[4 * d + 2] = al[:, d]
        rows[4 * d + 3] = al[:, d]
    rows[12] = ma
    rows[13] = ra
    rows[14:18] = 1.0
    return rows.astype(F32)


def _rhs_rows(cb):
    """[m,3] -> [18,m]; fraction rows encode the ORIGINAL candidate index."""
    b = cb.astype(np.int64)
    m = b.shape[0]
    bh, bl = b >> 3, b & 7
    nb2 = (b * b).sum(1)
    mb, rb = nb2 >> 8, nb2 & 255
    f = 2 * np.arange(m, dtype=np.int64) + 1
    fh, fl = f >> 6, f & 63
    r = np.zeros((18, m), np.float64)
    for d in range(3):
        r[4 * d + 0] = 128.0 * bh[:, d]
        r[4 * d + 1] = 16.0 * bl[:, d]
        r[4 * d + 2] = 16.0 * bh[:, d]
        r[4 * d + 3] = 2.0 * bl[:, d]
    r[12] = -256.0
    r[13] = -1.0
    r[14] = -256.0 * mb
    r[15] = -1.0 * rb
    r[16] = -(fh / 512.0)
    r[17] = -(fl / 32768.0)
    return r.astype(F32)


def _bf16(a):
    out = a.astype(BF16)
    assert np.array_equal(out.astype(F32), a)
    return out


def _scaled_feats(fb, w1, b1):
    fb = fb.astype(F32)
    z = fb @ w1.astype(F32) + b1.astype(F32)
    s = (1.0 / (1.0 + np.exp(-z, dtype=F32))).astype(F32)
    out = np.zeros((fb.shape[0], CPAD), F32)
    out[:, :C] = s * fb
    return out


def _kd_order(pts, leaf):
    out = []

    def rec(ids):
        if len(ids) <= leaf:
            out.append(ids)
            return
        p = pts[ids]
        dim = int(np.argmax(p.max(0) - p.min(0)))
        half = len(ids) // 2
        part = np.argpartition(p[:, dim], half)
        rec(ids[part[:half]])
        rec(ids[part[half:]])

    rec(np.arange(len(pts)))
    return np.concatenate(out)


def _plan_group(ca_g, cb_g):
    """k-d sort orders + per-half per-qtile surviving chunk lists."""
    pa = _kd_order(ca_g, 128)
    pb = _kd_order(cb_g, CHW)
    qa, qb = ca_g[pa], cb_g[pb]
    ct = qb.reshape(-1, CHW, 3)
    clo, chi = ct.min(1), ct.max(1)
    chunk_lists = []
    for h in range(2):
        qt = qa[h * QPC : (h + 1) * QPC].reshape(-1, 128, 3)
        qlo, qhi = qt.min(1), qt.max(1)
        lo = np.maximum(qlo[:, None, :], clo[None, :, :])
        hi = np.minimum(qhi[:, None, :], chi[None, :, :])
        gap = np.maximum(lo - hi, 0).astype(np.int64)
        keep = (gap**2).sum(-1) < 256
        chunk_lists.append([np.flatnonzero(keep[t]) for t in range(keep.shape[0])])
    return pa, pb, chunk_lists


def _make_caps(all_counts):
    nslots = len(all_counts[0])
    ranked = [sorted(c, reverse=True) for c in all_counts]
    return [max(1, max(r[t] for r in ranked)) for t in range(nslots)]


def _pack_core(chunks, caps, nch_total):
    order = np.argsort([-len(c) for c in chunks], kind="stable")
    slot_chunks = []
    for t, qt in enumerate(order):
        sel = list(chunks[qt])
        assert len(sel) <= caps[t]
        if len(sel) < caps[t]:
            selset = set(sel)
            pad = next(c for c in range(nch_total) if c not in selset)
            sel = sel + [pad] * (caps[t] - len(sel))
        slot_chunks.append(np.array(sel))
    return order, slot_chunks


# ---------------------------------------------------------------- device
def _build_program(nq, nb, caps, bqt=8):
    import concourse.tile as tile
    from concourse import bacc, mybir
    from concourse import library_config
    from concourse.tile_rust import add_dep_helper

    nqt = nq // 128
    nbt = nqt // bqt
    assert nqt % bqt == 0 and len(caps) == nqt
    f32, bf16, i16 = mybir.dt.float32, mybir.dt.bfloat16, mybir.dt.int16
    TWO23 = float(2.0**23)
    AL = mybir.AluOpType
    AF = mybir.ActivationFunctionType
    totcol = sum(c * CHW for c in caps)
    col_off = np.cumsum([0] + [c * CHW for c in caps])
    # rank of (bt, j) under round-robin dealing: r = nbt*j + bt
    rank = lambda bt, j: nbt * j + bt
    wmax = max(caps) * CHW
    npsmax = (wmax + 2047) // 2048
    nidx = bqt * TOPK * 128  # gather rows per bt-group

    nc = bacc.Bacc(
        "TRN2", target_bir_lowering=False, debug=False, num_swdge_queues=2
    )
    LT1 = nc.dram_tensor("lt1", [18, nq], bf16, kind="ExternalInput")
    RS = nc.dram_tensor("rs", [18, totcol], bf16, kind="ExternalInput")
    FBP = nc.dram_tensor("fbp", [nb, CPAD], f32, kind="ExternalInput")
    TMP = nc.dram_tensor("tmp", [nq, C], f32, kind="ExternalOutput")

    with tile.TileContext(nc) as tc:
        with (
            tc.tile_pool(name="const", bufs=1) as constp,
            tc.tile_pool(name="rstr", bufs=3) as rstrp,
            tc.tile_pool(name="cand", bufs=2) as candp,
            tc.tile_pool(name="small", bufs=3) as smallp,
            tc.tile_pool(name="wrap", bufs=2) as wrapp,
            tc.tile_pool(name="gath", bufs=3) as gathp,
            tc.tile_pool(name="prod", bufs=2) as prodp,
            tc.tile_pool(name="acc", bufs=2) as accp,
            tc.tile_pool(name="psum", bufs=2, space="PSUM") as psp,
        ):
            lib_inst = nc.gpsimd.load_library(library_config.mlp)

            lt1_sb = constp.tile([18, nq], bf16)
            nc.sync.dma_start(lt1_sb[:], LT1[:])

            state = {}  # carries phase-C inputs from the previous bt

            def a_tile(bt, j, top8):
                r = rank(bt, j)
                w_t = caps[r] * CHW
                rsb = rstrp.tile([18, wmax], bf16, tag="rsb")
                nc.sync.dma_start(
                    rsb[:, :w_t], RS[:, col_off[r] : col_off[r] + w_t]
                )
                nps_t = (w_t + 2047) // 2048
                cand = candp.tile([128, npsmax * 8], f32, tag="cand")
                if nps_t < npsmax:
                    nc.vector.memset(cand[:], -1.0e9)
                for h in range(nps_t):
                    pw = min(2048, w_t - h * 2048)
                    ps = psp.tile([128, 2048], f32, tag="ps")
                    for cc in range(0, pw, 512):
                        mw = min(512, pw - cc)
                        nc.tensor.matmul(
                            ps[:, cc : cc + mw],
                            lt1_sb[:, r * 128 : (r + 1) * 128],
                            rsb[:, h * 2048 + cc : h * 2048 + cc + mw],
                            start=True,
                            stop=True,
                        )
                    # top-8 of the chunk straight from PSUM
                    nc.vector.max(cand[:, h * 8 : (h + 1) * 8], ps[:, :pw])
                nc.vector.max(top8[:, j, :], cand[:])

            def c_tile(bt, j, G, wgt, acc):
                prod = prodp.tile([128, TOPK, C], f32, tag="prod")
                for k in range(TOPK):
                    nc.scalar.mul(
                        prod[:, k, :], G[j // 4][:, (j % 4) * TOPK + k, :C],
                        wgt[:, j, k : k + 1],
                    )
                nc.vector.tensor_reduce(
                    acc[:, j, :],
                    prod[:].rearrange("p k c -> p c k"),
                    mybir.AxisListType.X,
                    AL.add,
                )

            def c_out(bt, acc):
                tmp_v = TMP[:].rearrange(
                    "(rj rb q) c -> rb q rj c", rj=bqt, rb=nbt, q=128
                )[bt]
                nc.scalar.dma_start(tmp_v, acc[:])

            def phase_d(bt, top8):
                # decode keys: top8 = -(d2 + frac), frac in (0, 0.5)
                t8 = top8[:].rearrange("p a b -> p (a b)")
                wide = [128, bqt * 8]
                r1t = smallp.tile(wide, f32, tag="r1t")
                nc.vector.tensor_scalar(r1t[:], t8, -1.0, TWO23, AL.mult, AL.add)
                rr = smallp.tile(wide, f32, tag="rr")  # = d2
                nc.vector.tensor_scalar(rr[:], r1t[:], -TWO23, 0.0, AL.add, AL.add)
                ttm = smallp.tile(wide, f32, tag="ttm")  # = -frac
                nc.vector.tensor_tensor(ttm[:], t8, rr[:], AL.add)
                jj = smallp.tile(wide, f32, tag="jj")  # = orig index
                nc.vector.tensor_scalar(
                    jj[:], ttm[:], -16384.0, -0.5, AL.mult, AL.add
                )
                jc = smallp.tile([128, bqt, 8], f32, tag="jc")
                nc.vector.tensor_scalar(
                    jc[:].rearrange("p a b -> p (a b)"), jj[:], 0.0, float(nb - 1),
                    AL.max, AL.min,
                )
                sq = smallp.tile(wide, f32, tag="sq")
                nc.scalar.sqrt(sq[:], rr[:])
                wgt = smallp.tile([128, bqt, 8], f32, tag="wgt")
                nc.scalar.activation(
                    wgt[:].rearrange("p a b -> p (a b)"), sq[:],
                    AF.Relu, bias=1.0, scale=-0.0625,
                )

                # i16 index image: value of (query q, slot s=j*5+k) must land
                # at wrap[q%16, s*8 + q//16], replicated to all 128 partitions.
                jci = smallp.tile([128, bqt, TOPK], i16, tag="jci")
                nc.vector.tensor_copy(jci[:], jc[:, :, 0:TOPK])
                wrap = wrapp.tile([128, nidx // 16], i16, tag="wrap")
                wrap3 = wrap[0:16, :].rearrange("p (s a) -> p s a", a=8)
                for a in range(8):
                    nc.scalar.dma_start(
                        wrap3[:, :, a],
                        jci[16 * a : 16 * (a + 1), :, :].rearrange(
                            "p s k -> p (s k)"
                        ),
                    )
                nc.scalar.dma_start(wrap[16:32, :], wrap[0:16, :])
                nc.scalar.dma_start(wrap[32:64, :], wrap[0:32, :])
                nc.scalar.dma_start(wrap[64:128, :], wrap[0:64, :])

                # two half-gathers (separate G tiles) so tail-phase C can
                # start on the first half while the second is in flight
                hidx = nidx // 2
                G = []
                for half in range(2):
                    Gh = gathp.tile(
                        [128, bqt * TOPK // 2, CPAD], f32, tag=f"G{half}"
                    )
                    g_inst = nc.gpsimd.dma_gather(
                        Gh[:], FBP[:],
                        wrap[:, half * (hidx // 16) : (half + 1) * (hidx // 16)],
                        hidx, hidx, CPAD,
                        single_packet=False, queue_num=(2 * bt + half) % 2,
                    )
                    add_dep_helper(
                        g_inst.ins, lib_inst.ins, True, "gather waits lib"
                    )
                    G.append(Gh)
                return G, wgt

            # software pipeline: iteration bt runs phase A+D of bt with phase
            # C of bt-2 interleaved at tile granularity (its gather is done).
            for bt in range(nbt + 2):
                if bt < nbt:
                    top8 = smallp.tile([128, bqt, 8], f32, tag="top8")
                prev = state.pop(bt - 2, None)
                if prev is not None:
                    acc = accp.tile([128, bqt, C], f32, tag="acc")
                for j in range(bqt):
                    if bt < nbt:
                        a_tile(bt, j, top8)
                    if prev is not None:
                        c_tile(bt - 2, j, *prev, acc)
                if prev is not None:
                    c_out(bt - 2, acc)
                if bt < nbt:
                    state[bt] = phase_d(bt, top8)

    nc.compile()
    return nc


# ---------------------------------------------------------------- driver
def _prepare(coords_a, coords_b, feats_b, w1, b1):
    """Plan, build/compile (cached by caps), and produce per-core inputs.

    Returns (nc, in_maps, row_maps): row_maps[c] maps each output row of
    core c to its original query row within the core's group.
    """
    plans = [_plan_group(coords_a[g], coords_b[g]) for g in range(B)]
    all_counts = []
    for g in range(B):
        for h in range(2):
            all_counts.append([len(x) for x in plans[g][2][h]])
    caps = _make_caps(all_counts)

    key = tuple(caps)
    if _CACHE.get("key") != key:
        _CACHE["nc"] = _build_program(QPC, NB, caps)
        _CACHE["key"] = key
    nc = _CACHE["nc"]

    in_maps, row_maps = [], []
    for g in range(B):
        pa, pb, chunk_lists = plans[g]
        fbp = _scaled_feats(feats_b[g], w1, b1)
        rb_sorted = np.ascontiguousarray(_rhs_rows(coords_b[g])[:, pb])
        for h in range(2):
            qids = pa[h * QPC : (h + 1) * QPC]
            my_q = coords_a[g][qids]
            order, slot_chunks = _pack_core(chunk_lists[h], caps, NB // CHW)
            lt = _lhs_rows(my_q)
            lt_slots = np.concatenate(
                [lt[:, t * 128 : (t + 1) * 128] for t in order], axis=1
            )
            rs = np.concatenate(
                [rb_sorted[:, c0 * CHW : (c0 + 1) * CHW]
                 for sel in slot_chunks for c0 in sel],
                axis=1,
            )
            row_maps.append(
                np.concatenate([qids[t * 128 : (t + 1) * 128] for t in order])
            )
            in_maps.append(
                {
                    "lt1": _bf16(lt_slots),
                    "rs": _bf16(np.ascontiguousarray(rs)),
                    "fbp": fbp,
                }
            )
    return nc, in_maps, row_maps


def kernel(coords_a, coords_b, feats_a, feats_b, w1, b1):
    from concourse.bass_utils import run_bass_kernel_spmd

    coords_a = np.asarray(coords_a)
    coords_b = np.asarray(coords_b)
    feats_a = np.asarray(feats_a, dtype=F32)
    feats_b = np.asarray(feats_b, dtype=F32)
    w1 = np.asarray(w1, dtype=F32)
    b1 = np.asarray(b1, dtype=F32)

    nc, in_maps, row_maps = _prepare(coords_a, coords_b, feats_b, w1, b1)
    res = run_bass_kernel_spmd(nc, in_maps, core_ids=list(range(NCORES)))

    out = np.empty((B, NA, 2 * C), F32)
    out[:, :, :C] = feats_a
    for c in range(NCORES):
        g = c // 2
        out[g][row_maps[c], C:] = res.results[c]["tmp"]
    return out


# revision 14
# speedup vs baseline: 2.0406x; 1.2123x over previous
"""DistMatch (retrieval_knn) Trainium2 kernel — 8-core SPMD, bbox-pruned.

Problem (per batch group b of 4): for each of 8192 query points (int coords
in [0,128)^3), find the 5 candidates (of 8192) with smallest clipped L2
distance (ties -> lowest index, exactly like jax.lax.top_k), and accumulate
sigmoid-gated, distance-weighted candidate features.

Sharding: data-parallel over groups x query halves — core c handles group
c//2 and half c%2 of that group's (k-d sorted) queries.

Method:
  * Exact integer algebra: key = d2 + (2*orig_idx+1)/32768 is computed
    bit-exactly by one K=18 bf16 matmul per <=512 candidate columns
    (integer decomposition of coords/norms into bf16-exact rows).
  * Host k-d sorts queries (tiles of 128) and candidates (chunks of 256)
    and drops (qtile, chunk) pairs whose bounding boxes are > 16 apart.
    A rank-max slot schedule keeps the SPMD program identical across cores.
  * Software-pipelined device program: per bt-group, phase A (matmul +
    VectorE top-8 straight from PSUM), phase D (decode + on-chip i16 index
    wrap build + GPSIMD dma_gather launch), phase C of the PREVIOUS group
    (ScalarE weighted products + VectorE reduce + output DMA). The gather
    of group k overlaps the matmul/max of group k+1, keeping GPSIMD (the
    serial ~10ns/row descriptor generator) saturated.
  * Ranked slots are dealt round-robin to bt-groups so each group carries
    ~1/4 of the candidate columns.
"""

import numpy as np
import ml_dtypes

B = 4
NA = 8192
NB = 8192
C = 112
CPAD = 128
TOPK = 5
NCORES = 8
QPC = NA // 2
CHW = 128  # candidate chunk width

BF16 = ml_dtypes.bfloat16
F32 = np.float32

_CACHE: dict = {}


# ---------------------------------------------------------------- host math
def _lhs_rows(ca):
    a = ca.astype(np.int64)
    ah, al = a >> 3, a & 7
    na2 = (a * a).sum(1)
    ma, ra = na2 >> 8, na2 & 255
    rows = np.zeros((18, a.shape[0]), np.float64)
    for d in range(3):
        rows[4 * d + 0] = ah[:, d]
        rows[4 * d + 1] = ah[:, d]
        rows[4 * d + 2] = al[:, d]
        rows[4 * d + 3] = al[:, d]
    rows[12] = ma
    rows[13] = ra
    rows[14:18] = 1.0
    return rows.astype(F32)


def _rhs_rows(cb):
    """[m,3] -> [18,m]; fraction rows encode the ORIGINAL candidate index."""
    b = cb.astype(np.int64)
    m = b.shape[0]
    bh, bl = b >> 3, b & 7
    nb2 = (b * b).sum(1)
    mb, rb = nb2 >> 8, nb2 & 255
    f = 2 * np.arange(m, dtype=np.int64) + 1
    fh, fl = f >> 6, f & 63
    r = np.zeros((18, m), np.float64)
    for d in range(3):
        r[4 * d + 0] = 128.0 * bh[:, d]
        r[4 * d + 1] = 16.0 * bl[:, d]
        r[4 * d + 2] = 16.0 * bh[:, d]
        r[4 * d + 3] = 2.0 * bl[:, d]
    r[12] = -256.0
    r[13] = -1.0
    r[14] = -256.0 * mb
    r[15] = -1.0 * rb
    r[16] = -(fh / 512.0)
    r[17] = -(fl / 32768.0)
    return r.astype(F32)


def _bf16(a):
    out = a.astype(BF16)
    assert np.array_equal(out.astype(F32), a)
    return out


def _scaled_feats(fb, w1, b1):
    fb = fb.astype(F32)
    z = fb @ w1.astype(F32) + b1.astype(F32)
    s = (1.0 / (1.0 + np.exp(-z, dtype=F32))).astype(F32)
    out = np.zeros((fb.shape[0], CPAD), F32)
    out[:, :C] = s * fb
    return out


def _kd_order(pts, leaf):
    out = []

    def rec(ids):
        if len(ids) <= leaf:
            out.append(ids)
            return
        p = pts[ids]
        dim = int(np.argmax(p.max(0) - p.min(0)))
        half = len(ids) // 2
        part = np.argpartition(p[:, dim], half)
        rec(ids[part[:half]])
        rec(ids[part[half:]])

    rec(np.arange(len(pts)))
    return np.concatenate(out)


def _plan_group(ca_g, cb_g):
    """k-d sort orders + per-half per-qtile surviving chunk lists."""
    pa = _kd_order(ca_g, 128)
    pb = _kd_order(cb_g, CHW)
    qa, qb = ca_g[pa], cb_g[pb]
    ct = qb.reshape(-1, CHW, 3)
    clo, chi = ct.min(1), ct.max(1)
    chunk_lists = []
    for h in range(2):
        qt = qa[h * QPC : (h + 1) * QPC].reshape(-1, 128, 3)
        qlo, qhi = qt.min(1), qt.max(1)
        lo = np.maximum(qlo[:, None, :], clo[None, :, :])
        hi = np.minimum(qhi[:, None, :], chi[None, :, :])
        gap = np.maximum(lo - hi, 0).astype(np.int64)
        keep = (gap**2).sum(-1) < 256
        chunk_lists.append([np.flatnonzero(keep[t]) for t in range(keep.shape[0])])
    return pa, pb, chunk_lists


def _make_caps(all_counts):
    nslots = len(all_counts[0])
    ranked = [sorted(c, reverse=True) for c in all_counts]
    return [max(1, max(r[t] for r in ranked)) for t in range(nslots)]


def _pack_core(chunks, caps, nch_total):
    order = np.argsort([-len(c) for c in chunks], kind="stable")
    slot_chunks = []
    for t, qt in enumerate(order):
        sel = list(chunks[qt])
        assert len(sel) <= caps[t]
        if len(sel) < caps[t]:
            selset = set(sel)
            pad = next(c for c in range(nch_total) if c not in selset)
            sel = sel + [pad] * (caps[t] - len(sel))
        slot_chunks.append(np.array(sel))
    return order, slot_chunks


# ---------------------------------------------------------------- device
def _build_program(nq, nb, caps, bqt=8):
    import concourse.tile as tile
    from concourse import bacc, mybir
    from concourse import library_config
    from concourse.tile_rust import add_dep_helper

    nqt = nq // 128
    nbt = nqt // bqt
    assert nqt % bqt == 0 and len(caps) == nqt
    f32, bf16, i16 = mybir.dt.float32, mybir.dt.bfloat16, mybir.dt.int16
    TWO23 = float(2.0**23)
    AL = mybir.AluOpType
    AF = mybir.ActivationFunctionType
    totcol = sum(c * CHW for c in caps)
    col_off = np.cumsum([0] + [c * CHW for c in caps])
    # rank of (bt, j) under round-robin dealing: r = nbt*j + bt
    rank = lambda bt, j: nbt * j + bt
    wmax = max(caps) * CHW
    npsmax = (wmax + 2047) // 2048
    nidx = bqt * TOPK * 128  # gather rows per bt-group

    nc = bacc.Bacc(
        "TRN2", target_bir_lowering=False, debug=False, num_swdge_queues=2
    )
    LT1 = nc.dram_tensor("lt1", [18, nq], bf16, kind="ExternalInput")
    RS = nc.dram_tensor("rs", [18, totcol], bf16, kind="ExternalInput")
    FBP = nc.dram_tensor("fbp", [nb, CPAD], f32, kind="ExternalInput")
    TMP = nc.dram_tensor("tmp", [nq, C], f32, kind="ExternalOutput")

    with tile.TileContext(nc) as tc:
        with (
            tc.tile_pool(name="const", bufs=1) as constp,
            tc.tile_pool(name="rstr", bufs=3) as rstrp,
            tc.tile_pool(name="cand", bufs=2) as candp,
            tc.tile_pool(name="small", bufs=3) as smallp,
            tc.tile_pool(name="wrap", bufs=2) as wrapp,
            tc.tile_pool(name="gath", bufs=3) as gathp,
            tc.tile_pool(name="prod", bufs=2) as prodp,
            tc.tile_pool(name="acc", bufs=2) as accp,
            tc.tile_pool(name="psum", bufs=2, space="PSUM") as psp,
        ):
            lib_inst = nc.gpsimd.load_library(library_config.mlp)

            lt1_sb = constp.tile([18, nq], bf16)
            nc.sync.dma_start(lt1_sb[:], LT1[:])

            state = {}  # carries phase-C inputs from the previous bt

            def a_tile(bt, j, top8):
                r = rank(bt, j)
                w_t = caps[r] * CHW
                rsb = rstrp.tile([18, wmax], bf16, tag="rsb")
                nc.sync.dma_start(
                    rsb[:, :w_t], RS[:, col_off[r] : col_off[r] + w_t]
                )
                nps_t = (w_t + 2047) // 2048
                cand = candp.tile([128, npsmax * 8], f32, tag="cand")
                if nps_t < npsmax:
                    nc.vector.memset(cand[:], -1.0e9)
                for h in range(nps_t):
                    pw = min(2048, w_t - h * 2048)
                    ps = psp.tile([128, 2048], f32, tag="ps")
                    for cc in range(0, pw, 512):
                        mw = min(512, pw - cc)
                        nc.tensor.matmul(
                            ps[:, cc : cc + mw],
                            lt1_sb[:, r * 128 : (r + 1) * 128],
                            rsb[:, h * 2048 + cc : h * 2048 + cc + mw],
                            start=True,
                            stop=True,
                        )
                    # top-8 of the chunk straight from PSUM
                    nc.vector.max(cand[:, h * 8 : (h + 1) * 8], ps[:, :pw])
                nc.vector.max(top8[:, j, :], cand[:])

            def c_tile(bt, j, G, wgt, acc):
                prod = prodp.tile([128, TOPK, C], f32, tag="prod")
                for k in range(TOPK):
                    nc.scalar.mul(
                        prod[:, k, :], G[j // 4][:, (j % 4) * TOPK + k, :C],
                        wgt[:, j, k : k + 1],
                    )
                nc.vector.tensor_reduce(
                    acc[:, j, :],
                    prod[:].rearrange("p k c -> p c k"),
                    mybir.AxisListType.X,
                    AL.add,
                )

            def c_out(bt, acc):
                tmp_v = TMP[:].rearrange(
                    "(rj rb q) c -> rb q rj c", rj=bqt, rb=nbt, q=128
                )[bt]
                nc.scalar.dma_start(tmp_v, acc[:])

            def d_half(bt, half, top8, wgt):
                # decode keys for tiles j in [half*bqt/2, (half+1)*bqt/2):
                # top8 = -(d2 + frac), frac in (0, 0.5)
                hq = bqt // 2
                jsl = slice(half * hq, (half + 1) * hq)
                t8 = top8[:, jsl, :].rearrange("p a b -> p (a b)")
                wide = [128, hq * 8]
                r1t = smallp.tile(wide, f32, tag=f"r1t{half}")
                nc.vector.tensor_scalar(r1t[:], t8, -1.0, TWO23, AL.mult, AL.add)
                rr = smallp.tile(wide, f32, tag=f"rr{half}")  # = d2
                nc.vector.tensor_scalar(rr[:], r1t[:], -TWO23, 0.0, AL.add, AL.add)
                ttm = smallp.tile(wide, f32, tag=f"ttm{half}")  # = -frac
                nc.vector.tensor_tensor(ttm[:], t8, rr[:], AL.add)
                jj = smallp.tile(wide, f32, tag=f"jj{half}")  # = orig index
                nc.vector.tensor_scalar(
                    jj[:], ttm[:], -16384.0, -0.5, AL.mult, AL.add
                )
                jc = smallp.tile([128, hq, 8], f32, tag=f"jc{half}")
                nc.vector.tensor_scalar(
                    jc[:].rearrange("p a b -> p (a b)"), jj[:], 0.0, float(nb - 1),
                    AL.max, AL.min,
                )
                sq = smallp.tile(wide, f32, tag=f"sq{half}")
                nc.scalar.sqrt(sq[:], rr[:])
                nc.scalar.activation(
                    wgt[:, jsl, :].rearrange("p a b -> p (a b)"), sq[:],
                    AF.Relu, bias=1.0, scale=-0.0625,
                )

                # i16 index image: value of (query q, slot s=j*5+k) must land
                # at wrap[q%16, s*8 + q//16]; queues 0/1 read partitions 0-63.
                hidx = hq * TOPK * 128
                jci = smallp.tile([128, hq, TOPK], i16, tag=f"jci{half}")
                nc.vector.tensor_copy(jci[:], jc[:, :, 0:TOPK])
                wrap = wrapp.tile([64, hidx // 16], i16, tag=f"wrap{half}")
                wrap3 = wrap[0:16, :].rearrange("p (s a) -> p s a", a=8)
                for a in range(8):
                    nc.sync.dma_start(
                        wrap3[:, :, a],
                        jci[16 * a : 16 * (a + 1), :, :].rearrange(
                            "p s k -> p (s k)"
                        ),
                    )
                nc.sync.dma_start(wrap[16:32, :], wrap[0:16, :])
                nc.sync.dma_start(wrap[32:64, :], wrap[0:32, :])
                return wrap

            def launch_gather(bt, half, wrap):
                hq = bqt // 2
                hidx = hq * TOPK * 128
                Gh = gathp.tile([128, hq * TOPK, CPAD], f32, tag=f"G{half}")
                g_inst = nc.gpsimd.dma_gather(
                    Gh[:], FBP[:], wrap[:], hidx, hidx, CPAD,
                    single_packet=False, queue_num=half,
                )
                add_dep_helper(
                    g_inst.ins, lib_inst.ins, True, "gather waits lib"
                )
                return Gh

            # software pipeline: iteration bt runs phase A+D of bt (decode +
            # gather launched per half-group) with phase C of bt-2
            # interleaved at tile granularity (its gathers are done).
            for bt in range(nbt + 2):
                if bt < nbt:
                    top8 = smallp.tile([128, bqt, 8], f32, tag="top8")
                    wgt = smallp.tile([128, bqt, 8], f32, tag="wgt")
                    wraps = []
                prev = state.pop(bt - 2, None)
                if prev is not None:
                    acc = accp.tile([128, bqt, C], f32, tag="acc")
                for j in range(bqt):
                    if bt < nbt:
                        a_tile(bt, j, top8)
                        if j % (bqt // 2) == bqt // 2 - 1:
                            wraps.append(d_half(bt, j // (bqt // 2), top8, wgt))
                    if prev is not None:
                        c_tile(bt - 2, j, *prev, acc)
                if prev is not None:
                    c_out(bt - 2, acc)
                if bt < nbt:
                    # launch both half-gathers back-to-back: adjacent ready
                    # instructions run concurrently on distinct Q7 cpu pairs
                    G = [launch_gather(bt, h, w) for h, w in enumerate(wraps)]
                    state[bt] = (G, wgt)

    nc.compile()
    return nc


# ---------------------------------------------------------------- driver
def _prepare(coords_a, coords_b, feats_b, w1, b1):
    """Plan, build/compile (cached by caps), and produce per-core inputs.

    Returns (nc, in_maps, row_maps): row_maps[c] maps each output row of
    core c to its original query row within the core's group.
    """
    plans = [_plan_group(coords_a[g], coords_b[g]) for g in range(B)]
    all_counts = []
    for g in range(B):
        for h in range(2):
            all_counts.append([len(x) for x in plans[g][2][h]])
    caps = _make_caps(all_counts)

    key = tuple(caps)
    if _CACHE.get("key") != key:
        _CACHE["nc"] = _build_program(QPC, NB, caps)
        _CACHE["key"] = key
    nc = _CACHE["nc"]

    in_maps, row_maps = [], []
    for g in range(B):
        pa, pb, chunk_lists = plans[g]
        fbp = _scaled_feats(feats_b[g], w1, b1)
        rb_sorted = np.ascontiguousarray(_rhs_rows(coords_b[g])[:, pb])
        for h in range(2):
            qids = pa[h * QPC : (h + 1) * QPC]
            my_q = coords_a[g][qids]
            order, slot_chunks = _pack_core(chunk_lists[h], caps, NB // CHW)
            lt = _lhs_rows(my_q)
            lt_slots = np.concatenate(
                [lt[:, t * 128 : (t + 1) * 128] for t in order], axis=1
            )
            rs = np.concatenate(
                [rb_sorted[:, c0 * CHW : (c0 + 1) * CHW]
                 for sel in slot_chunks for c0 in sel],
                axis=1,
            )
            row_maps.append(
                np.concatenate([qids[t * 128 : (t + 1) * 128] for t in order])
            )
            in_maps.append(
                {
                    "lt1": _bf16(lt_slots),
                    "rs": _bf16(np.ascontiguousarray(rs)),
                    "fbp": fbp,
                }
            )
    return nc, in_maps, row_maps


def kernel(coords_a, coords_b, feats_a, feats_b, w1, b1):
    from concourse.bass_utils import run_bass_kernel_spmd

    coords_a = np.asarray(coords_a)
    coords_b = np.asarray(coords_b)
    feats_a = np.asarray(feats_a, dtype=F32)
    feats_b = np.asarray(feats_b, dtype=F32)
    w1 = np.asarray(w1, dtype=F32)
    b1 = np.asarray(b1, dtype=F32)

    nc, in_maps, row_maps = _prepare(coords_a, coords_b, feats_b, w1, b1)
    res = run_bass_kernel_spmd(nc, in_maps, core_ids=list(range(NCORES)))

    out = np.empty((B, NA, 2 * C), F32)
    out[:, :, :C] = feats_a
    for c in range(NCORES):
        g = c // 2
        out[g][row_maps[c], C:] = res.results[c]["tmp"]
    return out
